# revision 62
# baseline (speedup 1.0000x reference)
"""Trainium2 Bass kernel for causal multi-head attention with RoPE.

Problem: x[2,2048,2048], 16 heads, head_dim 128, fp32.
  q/k/v = x @ w{q,k,v}^T ; RoPE on q,k ; causal softmax(q k^T / sqrt(128)) @ v ; out @ wo^T

Sharding: Megatron tensor-parallel over heads - 2 heads per core on 8 cores.
Each core computes a partial y (its 2 heads' contribution through wo); the host
sums the 8 partials.  No device collectives.

Per-core design (v2, all matmul operands bf16; fp8 was tested on CPU and
fails the 2e-2 gate at ~4e-2):
  - x pre-transposed/tiled bf16 on host; q^T,k^T computed feature-major,
    v token-major.  RoPE rotate-half built with a tiny constant matmul on
    the PE (prot = R^T q) so the DVE does only 3 tensor_tensor ops per
    RoPE application instead of 5.
  - scores computed transposed S^T[key,q] = kT.T @ qT, one K=128 pass.
    Causal handled at 128-granularity: for the 4 diagonal-crossing key
    tiles the query slice is trimmed to [128*mi : 512], which makes the
    score/exp/AV work exactly the lower-triangular block count; the
    remaining triangle uses a single [128,512] 0/1 bf16 mask (prefix
    slices of it serve every trim width).
  - softmax without max-subtraction (scores bounded, exp safe in fp32):
    P^T = exp(S^T/sqrt(128)) on ACT, bf16.
  - row sums: P tiles are accumulated into an f32 SBUF tile on the Pool
    engine (which is otherwise idle); one ones[128,128]-stationary matmul
    per (qt,h) then yields the per-query sums replicated across all 128
    PSUM partitions, so 1/r comes from one fast [128,512]
    reciprocal_approx_fast and feeds a plain tensor_tensor multiply - no
    partition_broadcast, no slow single-partition reciprocal.
  - o^T = v.T @ P^T accumulated in PSUM; normalization deferred by one
    half-unit so the PE never waits on the Pool accumulation.
  - y rows = (o_norm^T).T @ woT written bf16 (host sums partials in
    fp64); PSUM->SBUF y copies alternate ACT/DVE to balance engines.
  - phase interleaving: attention of (b,qt) is emitted as soon as its
    token tiles are projected, filling the projection-phase gaps.
"""

import math
import sys

sys.path.insert(0, "/opt/trn_rl_repo")

import ml_dtypes  # noqa: E402
import numpy as np  # noqa: E402

P = 128
D = 2048
HD = 128  # head dim
B = 2
T = 2048
TOK = B * T  # 4096
NCORES = 8
HPC = 2  # heads per core
DC = HPC * HD  # 256 dims per core
CCHUNKS = D // P  # 16 contraction chunks
CPAIRS = CCHUNKS // 2  # 8 chunk pairs (one DMA each)
TT = TOK // 512  # 8 token tiles of 512
QT = T // 512  # 4 query tiles per batch
KT_PER_Q = 512 // P  # 4 key tiles per query tile

_CACHE = {}


def _build_nc():
    import concourse.bacc as bacc
    import concourse.mybir as mybir
    import concourse.tile as tile

    f32 = mybir.dt.float32
    f32r = mybir.dt.float32r
    bf16 = mybir.dt.bfloat16

    nc = bacc.Bacc("TRN2", target_bir_lowering=False, debug=False, num_devices=NCORES)

    # x pre-tiled on host: [tt, cpair, 128, 2, 512] bf16, contiguous per pair
    xTt = nc.dram_tensor("xTt", [TT, CPAIRS, P, 2, 512], bf16,
                         kind="ExternalInput").ap()
    cosT = nc.dram_tensor("cosT", [HD, TOK], bf16, kind="ExternalInput").ap()
    sinT = nc.dram_tensor("sinT", [HD, TOK], bf16, kind="ExternalInput").ap()
    wqT = nc.dram_tensor("wqT", [D, DC], bf16, kind="ExternalInput").ap()
    wkT = nc.dram_tensor("wkT", [D, DC], bf16, kind="ExternalInput").ap()
    wvT = nc.dram_tensor("wvT", [D, DC], bf16, kind="ExternalInput").ap()
    woT = nc.dram_tensor("woT", [DC, D], bf16, kind="ExternalInput").ap()
    y = nc.dram_tensor("y", [TOK, D], bf16, kind="ExternalOutput").ap()

    inv_sqrt_hd = 1.0 / math.sqrt(HD)

    with tile.TileContext(nc) as tc:
        with (
            tc.tile_pool(name="consts", bufs=1) as consts,
            tc.tile_pool(name="wpool", bufs=1) as wpool,
            tc.tile_pool(name="qkv", bufs=1) as qkv,
            tc.tile_pool(name="xp", bufs=10) as xp,
            tc.tile_pool(name="csp", bufs=2) as csp,
            tc.tile_pool(name="ropep", bufs=2) as ropep,
            tc.tile_pool(name="pap", bufs=4) as pap,
            tc.tile_pool(name="ptp", bufs=4) as ptp,
            tc.tile_pool(name="rrp", bufs=2) as rrp,
            tc.tile_pool(name="onp", bufs=3) as onp,
            tc.tile_pool(name="ysp", bufs=3) as ysp,
            tc.tile_pool(name="ps", bufs=8, space="PSUM") as ps,
        ):
            # ---- constants (tiles here; init emitted inside tile 0 so the
            # Pool-queue memsets don't delay the first weight DMAs) ----
            # single causal 0/1 bf16 mask: keep where q_local - key_local >= 0.
            # Diagonal tile mi uses mask[:, :512-128*mi] against the trimmed
            # query slice starting at 128*mi.
            mask = consts.tile([P, 512], bf16, tag="mask")
            ones_sq = consts.tile([P, P], bf16, tag="ones_sq")

            def emit_consts():
                nc.gpsimd.memset(mask[:], 1.0)
                nc.gpsimd.affine_select(
                    out=mask[:], in_=mask[:], compare_op=mybir.AluOpType.is_ge,
                    fill=0.0, base=0, channel_multiplier=-1, pattern=[[1, 512]],
                )
                nc.gpsimd.memset(ones_sq[:], 1.0)

            # ---- resident weights (DMAs staggered into tile 0's loop) ----
            wq_t = wpool.tile([P, CCHUNKS, DC], bf16, tag="wq")
            wk_t = wpool.tile([P, CCHUNKS, DC], bf16, tag="wk")
            wv_t = wpool.tile([P, CCHUNKS, DC], bf16, tag="wv")
            wo_t = wpool.tile([P, HPC, D], bf16, tag="wo")

            def emit_w_pair(cp):
                # weight traffic rides the (otherwise idle) Pool DGE queue so
                # it never delays the x-tile stream on the sync queue
                csl = slice(2 * cp, 2 * cp + 2)
                for wt, wdram in ((wq_t, wqT), (wk_t, wkT), (wv_t, wvT)):
                    nc.gpsimd.dma_start(
                        wt[:, csl, :],
                        wdram.rearrange("(co ci) d -> ci co d", ci=P)[:, csl, :])

            # ---- resident activations, one tile per 512-token block so the
            # dependency tracker keeps attention reads precise (a single big
            # tile accumulates 100+ writers and degrades to coarse deps that
            # serialize each attention unit behind the newest tile's RoPE) ----
            qT_ts = [qkv.tile([P, HPC, 512], bf16, tag=f"qT{t}", name=f"qT{t}")
                     for t in range(TT)]
            kT_ts = [qkv.tile([P, HPC, 512], bf16, tag=f"kT{t}", name=f"kT{t}")
                     for t in range(TT)]
            v_ts = [qkv.tile([P, 4, DC], bf16, tag=f"v{t}", name=f"v{t}")
                    for t in range(TT)]

            # ---- phase 1 tile body: projections + RoPE ----
            def emit_tile(tt, mid_cb=None):
                tsl = slice(tt * 512, (tt + 1) * 512)
                qT_t, kT_t, v_t = qT_ts[tt], kT_ts[tt], v_ts[tt]
                cos_t = csp.tile([P, 512], bf16, tag="cos")
                nc.scalar.dma_start(cos_t[:], cosT[:, tsl])
                sin_t = csp.tile([P, 512], bf16, tag="sin")
                nc.scalar.dma_start(sin_t[:], sinT[:, tsl])

                # --- pass A: q/k projections (4 PSUM banks), x tiles retained
                # in SBUF for the v pass ---
                pq = [ps.tile([P, 512], f32, tag="ps", name=f"pq{i}") for i in range(HPC)]
                pk = [ps.tile([P, 512], f32, tag="ps", name=f"pk{i}") for i in range(HPC)]
                xts = []
                for cp in range(CPAIRS):
                    if tt == 0 and cp == 0:
                        emit_w_pair(0)
                        emit_w_pair(1)
                        emit_consts()
                    if tt == 0 and cp + 2 < CPAIRS:
                        emit_w_pair(cp + 2)
                    xt = xp.tile([P, 2, 512], bf16, tag="x")
                    nc.sync.dma_start(xt[:], xTt[tt, cp])
                    xts.append(xt)
                    for j in range(2):
                        c = 2 * cp + j
                        st, sp = (c == 0), (c == CCHUNKS - 1)
                        xj = xt[:, j, :]
                        for h in range(HPC):
                            dsl = slice(h * HD, (h + 1) * HD)
                            nc.tensor.matmul(pq[h][:], wq_t[:, c, dsl], xj,
                                             start=st, stop=sp)
                            nc.tensor.matmul(pk[h][:], wk_t[:, c, dsl], xj,
                                             start=st, stop=sp)

                # evacuate q/k PSUM first so all banks are free for the
                # mid-tile attention unit.  On DVE (Pool has no PSUM route):
                # ACT must stay clear for the attention exps - a copy queued
                # ahead of them delays the first AV by ~3us.  These copies
                # only gate the RoPE, which has half a slot of slack.
                for h in range(HPC):
                    nc.vector.tensor_copy(qT_t[:, h, :], pq[h][:])
                    nc.vector.tensor_copy(kT_t[:, h, :], pk[h][:])

                # --- pass B: v projections (2 PSUM banks, x from SBUF),
                # interleaved one (cp,j) step per two attention kts so the
                # in-order PE queue always has wait-free work while ACT runs
                # the exps (score+AV per kt is ~430ns of PE against ~690ns of
                # exp; one v step adds ~460ns of filler). ---
                pv = [ps.tile([P, 512], f32, tag="ps", name=f"pv{i}") for i in range(2)]

                def vstep_emit(cp, j):
                    c = 2 * cp + j
                    st, sp = (c == 0), (c == CCHUNKS - 1)
                    for s4 in range(4):
                        half = s4 % 2
                        nc.tensor.matmul(
                            pv[s4 // 2][:, half * DC:(half + 1) * DC],
                            xts[cp][:, j, s4 * P:(s4 + 1) * P],
                            wv_t[:, c, :],
                            start=st and half == 0, stop=sp,
                            skip_group_check=half == 1)

                vqueue = [(cp, j) for cp in range(CPAIRS) for j in range(2)]
                vidx = [0]

                def vstep():
                    if vidx[0] < len(vqueue):
                        vstep_emit(*vqueue[vidx[0]])
                        vidx[0] += 1
                        return True
                    return False

                # mid-tile: the previous slot's attention unit, v steps woven
                # between its kts; its DVE work queues ahead of this RoPE
                if mid_cb is not None:
                    mid_cb(vstep)
                while vidx[0] < len(vqueue):
                    vstep()

                # RoPE in place, all-SBUF bf16 (DVE 2x path).  Partition-
                # shifting is only legal on copy-class ops, so swap halves
                # with two copies; sinT rows 0:64 are pre-negated on the host
                # so one aligned multiply finishes rotate-half:
                #   rot = swap_halves(raw);  rot *= sinN;  dst = raw*cos + rot
                for dst_t in (qT_t, kT_t):
                    for h in range(HPC):
                        dst = dst_t[:, h, :]
                        rot = ropep.tile([P, 512], bf16, tag="rot")
                        nc.vector.tensor_copy(rot[0:64, :], dst[64:128, :])
                        nc.vector.tensor_copy(rot[64:128, :], dst[0:64, :])
                        nc.vector.tensor_mul(out=rot[:], in0=rot[:], in1=sin_t[:])
                        nc.vector.tensor_mul(out=dst, in0=dst, in1=cos_t[:])
                        nc.vector.tensor_add(out=dst, in0=dst, in1=rot[:])

                for s4 in range(4):
                    half = s4 % 2
                    nc.scalar.copy(v_t[:, s4, :],
                                   pv[s4 // 2][:, half * DC:(half + 1) * DC])

            # ---- phase 2: attention + output projection ----
            pending_y = []
            pending_norm = []

            def emit_norm(pacc, onorm, h):
                # ones[128,128]-stationary puts the per-query sums on every
                # PSUM partition: 1/r needs no partition broadcast.  Deferred
                # one half-unit so the PE never waits on the DVE P-sums.
                pr = ps.tile([P, 512], f32, tag="ps", name="pr")
                nc.tensor.matmul(pr[:], ones_sq[:], pacc[:], start=True,
                                 stop=True)
                rr = rrp.tile([P, 512], f32, tag="rr")
                nc.vector.reciprocal_approx_fast(out=rr[:], in_=pr[:])
                nc.vector.tensor_mul(out=onorm[:, h, :], in0=onorm[:, h, :],
                                     in1=rr[:])

            def make_yproj_steps(onorm, b, qt):
                # one step per (s4,dout): 2 matmuls + a PSUM->SBUF copy, DMA
                # after the last dout.  Steps are woven between attention kts
                # as PE filler.
                steps = []
                state = {}

                def step(s4, dout):
                    if dout == 0:
                        state[s4] = ysp.tile([P, D], bf16, tag="ystage",
                                             name="ystage")
                    ystage = state[s4]
                    py = ps.tile([P, 512], f32, tag="ps", name="py")
                    for h in range(HPC):
                        nc.tensor.matmul(
                            py[:],
                            onorm[:, h, s4 * P:(s4 + 1) * P],
                            wo_t[:, h, dout * 512:(dout + 1) * 512],
                            start=(h == 0), stop=(h == HPC - 1))
                    if dout % 2 == 0:
                        nc.scalar.copy(ystage[:, dout * 512:(dout + 1) * 512],
                                       py[:])
                    else:
                        nc.vector.tensor_copy(
                            ystage[:, dout * 512:(dout + 1) * 512], py[:])
                    if dout == 3:
                        r0 = b * T + qt * 512 + s4 * P
                        # y rides the Pool DGE queue: its semaphore waits
                        # would otherwise block x prefetch on the sync queue
                        nc.gpsimd.dma_start(y[r0:r0 + P, :], ystage[:])

                for s4 in range(4):
                    for dout in range(4):
                        steps.append(lambda s4=s4, dout=dout: step(s4, dout))
                return steps

            def emit_yproj(onorm, b, qt):
                for s in make_yproj_steps(onorm, b, qt):
                    s()

            def emit_attn(b, qt, vstep=None):
                qtile = qT_ts[b * QT + qt]
                nkt = KT_PER_Q * (qt + 1)
                onorm = onp.tile([P, HPC, 512], bf16, tag="onorm")
                # filler queue: this slot's remaining v-projection steps, then
                # the previous unit's output projection.  yproj(u-1) steps may
                # only be emitted in the h1 half: its h1 normalization is
                # emitted at the end of our h0 half, and a PE step queued
                # before it while waiting on it would deadlock the queue.
                ysteps = []
                if pending_y:
                    ysteps = make_yproj_steps(*pending_y.pop(0))
                yidx = [0]

                def filler(h):
                    if vstep is not None and vstep():
                        return
                    if h == 1 and yidx[0] < len(ysteps):
                        ysteps[yidx[0]]()
                        yidx[0] += 1
                for h in range(HPC):
                    po = ps.tile([P, 512], f32, tag="ps", name="po")
                    pacc = pap.tile([P, 512], bf16, tag="pacc")

                    def emit_score(kt, h=h):
                        mi = kt - KT_PER_Q * qt  # >=0 on the diagonal
                        q0 = P * mi if mi > 0 else 0
                        free = 512 - q0
                        kt_t = kT_ts[b * QT + kt // 4]
                        k0 = (kt % 4) * P
                        pscore = ps.tile([P, 512], f32, tag="ps", name="pscore")
                        nc.tensor.matmul(pscore[:, :free],
                                         kt_t[:, h, k0:k0 + P],
                                         qtile[:, h, q0:512],
                                         start=True, stop=True)
                        ptile = ptp.tile([P, 512], bf16, tag="pt", name="ptile")
                        nc.scalar.activation(ptile[:, :free], pscore[:, :free],
                                             mybir.ActivationFunctionType.Exp,
                                             scale=inv_sqrt_hd)
                        if mi >= 0:
                            nc.vector.tensor_mul(out=ptile[:, :free],
                                                 in0=ptile[:, :free],
                                                 in1=mask[:, :free])
                        return ptile, q0, free

                    # kt loop pipelined two deep so the PE has wait-free score
                    # work while the exp(+mask) chain of earlier kts completes.
                    tiles = {}
                    for kt in range(min(2, nkt)):
                        tiles[kt] = emit_score(kt)
                    # pre-pop filler so the PE has work while the first
                    # exp(+mask) completes
                    filler(h)
                    filler(h)
                    for kt in range(nkt):
                        if kt + 2 < nkt:
                            tiles[kt + 2] = emit_score(kt + 2)
                        ptile, q0, free = tiles.pop(kt)
                        st, sp = (kt == 0), (kt == nkt - 1)
                        nc.tensor.matmul(po[:, q0:512],
                                         v_ts[b * QT + kt // 4][:, kt % 4,
                                             h * HD:(h + 1) * HD],
                                         ptile[:, :free], start=st, stop=sp)
                        # P accumulated on DVE (bf16 2x path) for the row
                        # sums; not in the AV critical path
                        if kt == 0:
                            nc.vector.tensor_copy(pacc[:], ptile[:])
                        else:
                            nc.vector.tensor_add(out=pacc[:, q0:512],
                                                 in0=pacc[:, q0:512],
                                                 in1=ptile[:, :free])
                        filler(h)
                    nc.scalar.copy(onorm[:, h, :], po[:])
                    pending_norm.append((pacc, onorm, h))
                    if len(pending_norm) > 1:
                        emit_norm(*pending_norm.pop(0))

                while yidx[0] < len(ysteps):
                    ysteps[yidx[0]]()
                    yidx[0] += 1
                pending_y.append((onorm, b, qt))

            # ---- schedule: attention unit (b,qt) = tile b*4+qt is emitted
            # mid-way through the NEXT tile's emission (see emit_tile) ----
            for tt in range(TT):
                if tt >= 1:
                    b, qt = divmod(tt - 1, QT)
                    emit_tile(tt, mid_cb=lambda vs, b=b, qt=qt: emit_attn(b, qt, vs))
                else:
                    emit_tile(tt)
                if tt == 1:
                    for h in range(HPC):
                        nc.scalar.dma_start(
                            wo_t[:, h, :],
                            woT.rearrange("(ko ki) n -> ki ko n", ki=P)[:, h, :])
            emit_attn(1, 3)
            for args in pending_norm:
                emit_norm(*args)
            for args in pending_y:
                emit_yproj(*args)

    nc.compile()
    return nc


def get_nc():
    if "nc" not in _CACHE:
        _CACHE["nc"] = _build_nc()
    return _CACHE["nc"]


def make_in_maps(x, cos, sin, wq, wk, wv, wo):
    bf = ml_dtypes.bfloat16
    xT = x.reshape(TOK, D).T  # [D, TOK]
    # [D, TOK] -> [TT, cpair, ci, j, 512]
    xTt = np.ascontiguousarray(
        xT.reshape(CPAIRS, 2, P, TT, 512).transpose(3, 0, 2, 1, 4)).astype(bf)
    cosT = np.ascontiguousarray(cos.reshape(TOK, HD).T).astype(bf)
    # rows 0:64 negated: rot_half contributes -x2*sin there (see kernel RoPE)
    sinT = np.ascontiguousarray(sin.reshape(TOK, HD).T).copy()
    sinT[0:64, :] *= -1.0
    sinT = sinT.astype(bf)
    in_maps = []
    for c in range(NCORES):
        dsl = slice(c * DC, (c + 1) * DC)
        in_maps.append({
            "xTt": xTt,
            "cosT": cosT,
            "sinT": sinT,
            "wqT": np.ascontiguousarray(wq[dsl, :].T).astype(bf),
            "wkT": np.ascontiguousarray(wk[dsl, :].T).astype(bf),
            "wvT": np.ascontiguousarray(wv[dsl, :].T).astype(bf),
            "woT": np.ascontiguousarray(wo[:, dsl].T).astype(bf),
        })
    return in_maps


def kernel(x, cos, sin, wq, wk, wv, wo):
    from concourse.bass_utils import run_bass_kernel_spmd

    nc = get_nc()
    in_maps = make_in_maps(
        np.asarray(x, dtype=np.float32), np.asarray(cos, dtype=np.float32),
        np.asarray(sin, dtype=np.float32), np.asarray(wq, dtype=np.float32),
        np.asarray(wk, dtype=np.float32), np.asarray(wv, dtype=np.float32),
        np.asarray(wo, dtype=np.float32))
    res = run_bass_kernel_spmd(nc, in_maps, list(range(NCORES)))
    out = np.zeros((TOK, D), dtype=np.float64)
    for m in res.results:
        out += m["y"].astype(np.float64)
    return out.astype(np.float32).reshape(B, T, D)


# revision 66
# speedup vs baseline: 1.2173x; 1.2173x over previous
"""Trainium2 Bass kernel for causal multi-head attention with RoPE.

Problem: x[2,2048,2048], 16 heads, head_dim 128, fp32.
  q/k/v = x @ w{q,k,v}^T ; RoPE on q,k ; causal softmax(q k^T / sqrt(128)) @ v ; out @ wo^T

Sharding: Megatron tensor-parallel over heads - 2 heads per core on 8 cores.
Each core computes a partial y (its 2 heads' contribution through wo); the host
sums the 8 partials.  No device collectives.

Per-core design (v2, all matmul operands bf16; fp8 was tested on CPU and
fails the 2e-2 gate at ~4e-2):
  - x pre-transposed/tiled bf16 on host; q^T,k^T computed feature-major,
    v token-major.  RoPE rotate-half built with a tiny constant matmul on
    the PE (prot = R^T q) so the DVE does only 3 tensor_tensor ops per
    RoPE application instead of 5.
  - scores computed transposed S^T[key,q] = kT.T @ qT, one K=128 pass.
    Causal handled at 128-granularity: for the 4 diagonal-crossing key
    tiles the query slice is trimmed to [128*mi : 512], which makes the
    score/exp/AV work exactly the lower-triangular block count; the
    remaining triangle uses a single [128,512] 0/1 bf16 mask (prefix
    slices of it serve every trim width).
  - softmax without max-subtraction (scores bounded, exp safe in fp32):
    P^T = exp(S^T/sqrt(128)) on ACT, bf16.
  - row sums: P tiles are accumulated into an f32 SBUF tile on the Pool
    engine (which is otherwise idle); one ones[128,128]-stationary matmul
    per (qt,h) then yields the per-query sums replicated across all 128
    PSUM partitions, so 1/r comes from one fast [128,512]
    reciprocal_approx_fast and feeds a plain tensor_tensor multiply - no
    partition_broadcast, no slow single-partition reciprocal.
  - o^T = v.T @ P^T accumulated in PSUM; normalization deferred by one
    half-unit so the PE never waits on the Pool accumulation.
  - y rows = (o_norm^T).T @ woT written bf16 (host sums partials in
    fp64); PSUM->SBUF y copies alternate ACT/DVE to balance engines.
  - phase interleaving: attention of (b,qt) is emitted as soon as its
    token tiles are projected, filling the projection-phase gaps.
"""

import math
import sys

sys.path.insert(0, "/opt/trn_rl_repo")

import ml_dtypes  # noqa: E402
import numpy as np  # noqa: E402

P = 128
D = 2048
HD = 128  # head dim
B = 2
T = 2048
TOK = B * T  # 4096
NCORES = 8
HPC = 2  # heads per core
DC = HPC * HD  # 256 dims per core
CCHUNKS = D // P  # 16 contraction chunks
CPAIRS = CCHUNKS // 2  # 8 chunk pairs (one DMA each)
TT = TOK // 512  # 8 token tiles of 512
QT = T // 512  # 4 query tiles per batch
KT_PER_Q = 512 // P  # 4 key tiles per query tile

_CACHE = {}


def _build_nc():
    import concourse.bacc as bacc
    import concourse.mybir as mybir
    import concourse.tile as tile

    f32 = mybir.dt.float32
    f32r = mybir.dt.float32r
    bf16 = mybir.dt.bfloat16

    nc = bacc.Bacc("TRN2", target_bir_lowering=False, debug=False, num_devices=NCORES)

    # x pre-tiled on host: [tt, cpair, 128, 2, 512] bf16, contiguous per pair
    xTt = nc.dram_tensor("xTt", [TT, CPAIRS, P, 2, 512], bf16,
                         kind="ExternalInput").ap()
    cosT = nc.dram_tensor("cosT", [HD, TOK], bf16, kind="ExternalInput").ap()
    sinT = nc.dram_tensor("sinT", [HD, TOK], bf16, kind="ExternalInput").ap()
    wqT = nc.dram_tensor("wqT", [D, DC], bf16, kind="ExternalInput").ap()
    wkT = nc.dram_tensor("wkT", [D, DC], bf16, kind="ExternalInput").ap()
    wvT = nc.dram_tensor("wvT", [D, DC], bf16, kind="ExternalInput").ap()
    woT = nc.dram_tensor("woT", [DC, D], bf16, kind="ExternalInput").ap()
    y = nc.dram_tensor("y", [TOK, D], bf16, kind="ExternalOutput").ap()

    inv_sqrt_hd = 1.0 / math.sqrt(HD)

    with tile.TileContext(nc) as tc:
        with (
            tc.tile_pool(name="consts", bufs=1) as consts,
            tc.tile_pool(name="wpool", bufs=1) as wpool,
            tc.tile_pool(name="qkv", bufs=1) as qkv,
            tc.tile_pool(name="xp", bufs=10) as xp,
            tc.tile_pool(name="csp", bufs=2) as csp,
            tc.tile_pool(name="ropep", bufs=2) as ropep,
            tc.tile_pool(name="pap", bufs=4) as pap,
            tc.tile_pool(name="ptp", bufs=4) as ptp,
            tc.tile_pool(name="rrp", bufs=2) as rrp,
            tc.tile_pool(name="onp", bufs=3) as onp,
            tc.tile_pool(name="ysp", bufs=3) as ysp,
            tc.tile_pool(name="ps", bufs=8, space="PSUM") as ps,
        ):
            # ---- constants (tiles here; init emitted inside tile 0 so the
            # Pool-queue memsets don't delay the first weight DMAs) ----
            # single causal 0/1 bf16 mask: keep where q_local - key_local >= 0.
            # Diagonal tile mi uses mask[:, :512-128*mi] against the trimmed
            # query slice starting at 128*mi.
            mask = consts.tile([P, 512], bf16, tag="mask")
            ones_sq = consts.tile([P, P], bf16, tag="ones_sq")

            def emit_consts():
                nc.gpsimd.memset(mask[:], 1.0)
                nc.gpsimd.affine_select(
                    out=mask[:], in_=mask[:], compare_op=mybir.AluOpType.is_ge,
                    fill=0.0, base=0, channel_multiplier=-1, pattern=[[1, 512]],
                )
                nc.gpsimd.memset(ones_sq[:], 1.0)

            # ---- resident weights (DMAs staggered into tile 0's loop) ----
            wq_t = wpool.tile([P, CCHUNKS, DC], bf16, tag="wq")
            wk_t = wpool.tile([P, CCHUNKS, DC], bf16, tag="wk")
            wv_t = wpool.tile([P, CCHUNKS, DC], bf16, tag="wv")
            wo_t = wpool.tile([P, HPC, D], bf16, tag="wo")

            def emit_w_pair(cp):
                # weight traffic rides the (otherwise idle) Pool DGE queue so
                # it never delays the x-tile stream on the sync queue
                csl = slice(2 * cp, 2 * cp + 2)
                for wt, wdram in ((wq_t, wqT), (wk_t, wkT), (wv_t, wvT)):
                    nc.gpsimd.dma_start(
                        wt[:, csl, :],
                        wdram.rearrange("(co ci) d -> ci co d", ci=P)[:, csl, :])

            # ---- resident activations, one tile per 512-token block so the
            # dependency tracker keeps attention reads precise (a single big
            # tile accumulates 100+ writers and degrades to coarse deps that
            # serialize each attention unit behind the newest tile's RoPE) ----
            qT_ts = [qkv.tile([P, HPC, 512], bf16, tag=f"qT{t}", name=f"qT{t}")
                     for t in range(TT)]
            kT_ts = [qkv.tile([P, HPC, 512], bf16, tag=f"kT{t}", name=f"kT{t}")
                     for t in range(TT)]
            v_ts = [qkv.tile([P, 4, DC], bf16, tag=f"v{t}", name=f"v{t}")
                    for t in range(TT)]

            # ---- phase 1 tile body: projections + RoPE ----
            def emit_tile(tt, mid_cb=None):
                tsl = slice(tt * 512, (tt + 1) * 512)
                qT_t, kT_t, v_t = qT_ts[tt], kT_ts[tt], v_ts[tt]
                cos_t = csp.tile([P, 512], bf16, tag="cos")
                nc.scalar.dma_start(cos_t[:], cosT[:, tsl])
                sin_t = csp.tile([P, 512], bf16, tag="sin")
                nc.scalar.dma_start(sin_t[:], sinT[:, tsl])

                # --- pass A: q/k projections (4 PSUM banks), x tiles retained
                # in SBUF for the v pass ---
                pq = [ps.tile([P, 512], f32, tag="ps", name=f"pq{i}") for i in range(HPC)]
                pk = [ps.tile([P, 512], f32, tag="ps", name=f"pk{i}") for i in range(HPC)]
                xts = []
                for cp in range(CPAIRS):
                    if tt == 0 and cp == 0:
                        emit_w_pair(0)
                        emit_w_pair(1)
                        emit_consts()
                    if tt == 0 and cp + 2 < CPAIRS:
                        emit_w_pair(cp + 2)
                    xt = xp.tile([P, 2, 512], bf16, tag="x")
                    nc.sync.dma_start(xt[:], xTt[tt, cp])
                    xts.append(xt)
                    for j in range(2):
                        c = 2 * cp + j
                        st, sp = (c == 0), (c == CCHUNKS - 1)
                        xj = xt[:, j, :]
                        for h in range(HPC):
                            dsl = slice(h * HD, (h + 1) * HD)
                            nc.tensor.matmul(pq[h][:], wq_t[:, c, dsl], xj,
                                             start=st, stop=sp)
                            nc.tensor.matmul(pk[h][:], wk_t[:, c, dsl], xj,
                                             start=st, stop=sp)

                # evacuate q/k PSUM first so all banks are free for the
                # mid-tile attention unit.  On DVE (Pool has no PSUM route):
                # ACT must stay clear for the attention exps - a copy queued
                # ahead of them delays the first AV by ~3us.  These copies
                # only gate the RoPE, which has half a slot of slack.
                for h in range(HPC):
                    nc.vector.tensor_copy(qT_t[:, h, :], pq[h][:])
                    nc.vector.tensor_copy(kT_t[:, h, :], pk[h][:])

                # --- pass B: v projections (2 PSUM banks, x from SBUF),
                # interleaved one (cp,j) step per two attention kts so the
                # in-order PE queue always has wait-free work while ACT runs
                # the exps (score+AV per kt is ~430ns of PE against ~690ns of
                # exp; one v step adds ~460ns of filler). ---
                pv = [ps.tile([P, 512], f32, tag="ps", name=f"pv{i}") for i in range(2)]

                def vstep_emit(cp, j):
                    c = 2 * cp + j
                    st, sp = (c == 0), (c == CCHUNKS - 1)
                    for s4 in range(4):
                        half = s4 % 2
                        nc.tensor.matmul(
                            pv[s4 // 2][:, half * DC:(half + 1) * DC],
                            xts[cp][:, j, s4 * P:(s4 + 1) * P],
                            wv_t[:, c, :],
                            start=st and half == 0, stop=sp,
                            skip_group_check=half == 1)

                vqueue = [(cp, j) for cp in range(CPAIRS) for j in range(2)]
                vidx = [0]

                def vstep():
                    if vidx[0] < len(vqueue):
                        vstep_emit(*vqueue[vidx[0]])
                        vidx[0] += 1
                        return True
                    return False

                # mid-tile: the previous slot's attention unit, v steps woven
                # between its kts; its DVE work queues ahead of this RoPE
                if mid_cb is not None:
                    mid_cb(vstep)
                while vidx[0] < len(vqueue):
                    vstep()

                # RoPE in place, all-SBUF bf16 (DVE 2x path).  Partition-
                # shifting is only legal on copy-class ops, so swap halves
                # with two copies; sinT rows 0:64 are pre-negated on the host
                # so one aligned multiply finishes rotate-half:
                #   rot = swap_halves(raw);  rot *= sinN;  dst = raw*cos + rot
                for dst_t in (qT_t, kT_t):
                    for h in range(HPC):
                        dst = dst_t[:, h, :]
                        rot = ropep.tile([P, 512], bf16, tag="rot")
                        nc.vector.tensor_copy(rot[0:64, :], dst[64:128, :])
                        nc.vector.tensor_copy(rot[64:128, :], dst[0:64, :])
                        nc.vector.tensor_mul(out=rot[:], in0=rot[:], in1=sin_t[:])
                        nc.vector.tensor_mul(out=dst, in0=dst, in1=cos_t[:])
                        nc.vector.tensor_add(out=dst, in0=dst, in1=rot[:])

                for s4 in range(4):
                    half = s4 % 2
                    nc.scalar.copy(v_t[:, s4, :],
                                   pv[s4 // 2][:, half * DC:(half + 1) * DC])

            # ---- phase 2: attention + output projection ----
            pending_y = []
            pending_norm = []

            def emit_norm(pacc, onorm, h):
                # ones[128,128]-stationary puts the per-query sums on every
                # PSUM partition: 1/r needs no partition broadcast.  Deferred
                # one half-unit so the PE never waits on the DVE P-sums.
                pr = ps.tile([P, 512], f32, tag="ps", name="pr")
                nc.tensor.matmul(pr[:], ones_sq[:], pacc[:], start=True,
                                 stop=True)
                rr = rrp.tile([P, 512], f32, tag="rr")
                nc.vector.reciprocal_approx_fast(out=rr[:], in_=pr[:])
                nc.vector.tensor_mul(out=onorm[:, h, :], in0=onorm[:, h, :],
                                     in1=rr[:])

            def make_yproj_steps(onorm, b, qt):
                # one step per (s4,dout): 2 matmuls + a PSUM->SBUF copy, DMA
                # after the last dout.  Steps are woven between attention kts
                # as PE filler.
                steps = []
                state = {}

                def step(s4, dout):
                    if dout == 0:
                        state[s4] = ysp.tile([P, D], bf16, tag="ystage",
                                             name="ystage")
                    ystage = state[s4]
                    py = ps.tile([P, 512], f32, tag="ps", name="py")
                    for h in range(HPC):
                        nc.tensor.matmul(
                            py[:],
                            onorm[:, h, s4 * P:(s4 + 1) * P],
                            wo_t[:, h, dout * 512:(dout + 1) * 512],
                            start=(h == 0), stop=(h == HPC - 1))
                    if dout % 2 == 0:
                        nc.scalar.copy(ystage[:, dout * 512:(dout + 1) * 512],
                                       py[:])
                    else:
                        nc.vector.tensor_copy(
                            ystage[:, dout * 512:(dout + 1) * 512], py[:])
                    if dout == 3:
                        r0 = b * T + qt * 512 + s4 * P
                        # y rides the Pool DGE queue: its semaphore waits
                        # would otherwise block x prefetch on the sync queue
                        nc.gpsimd.dma_start(y[r0:r0 + P, :], ystage[:])

                for s4 in range(4):
                    for dout in range(4):
                        steps.append(lambda s4=s4, dout=dout: step(s4, dout))
                return steps

            def emit_yproj(onorm, b, qt):
                for s in make_yproj_steps(onorm, b, qt):
                    s()

            def emit_attn(b, qt, vstep=None):
                qtile = qT_ts[b * QT + qt]
                nkt = KT_PER_Q * (qt + 1)
                onorm = onp.tile([P, HPC, 512], bf16, tag="onorm")

                for h in range(HPC):
                    po = ps.tile([P, 512], f32, tag="ps", name="po")
                    pacc = pap.tile([P, 512], bf16, tag="pacc")

                    def emit_score(kt, h=h):
                        mi = kt - KT_PER_Q * qt  # >=0 on the diagonal
                        q0 = P * mi if mi > 0 else 0
                        free = 512 - q0
                        kt_t = kT_ts[b * QT + kt // 4]
                        k0 = (kt % 4) * P
                        pscore = ps.tile([P, 512], f32, tag="ps", name="pscore")
                        nc.tensor.matmul(pscore[:, :free],
                                         kt_t[:, h, k0:k0 + P],
                                         qtile[:, h, q0:512],
                                         start=True, stop=True)
                        ptile = ptp.tile([P, 512], bf16, tag="pt", name="ptile")
                        nc.scalar.activation(ptile[:, :free], pscore[:, :free],
                                             mybir.ActivationFunctionType.Exp,
                                             scale=inv_sqrt_hd)
                        if mi >= 0:
                            nc.vector.tensor_mul(out=ptile[:, :free],
                                                 in0=ptile[:, :free],
                                                 in1=mask[:, :free])
                        return ptile, q0, free

                    # kt loop pipelined two deep so the PE has wait-free score
                    # work while the exp(+mask) chain of earlier kts completes.
                    tiles = {}
                    for kt in range(min(2, nkt)):
                        tiles[kt] = emit_score(kt)
                    # pre-pop filler so the PE has work while the first
                    # exp(+mask) completes
                    if vstep is not None:
                        vstep()
                        vstep()
                    for kt in range(nkt):
                        if kt + 2 < nkt:
                            tiles[kt + 2] = emit_score(kt + 2)
                        ptile, q0, free = tiles.pop(kt)
                        st, sp = (kt == 0), (kt == nkt - 1)
                        nc.tensor.matmul(po[:, q0:512],
                                         v_ts[b * QT + kt // 4][:, kt % 4,
                                             h * HD:(h + 1) * HD],
                                         ptile[:, :free], start=st, stop=sp)
                        # P accumulated on DVE (bf16 2x path) for the row
                        # sums; not in the AV critical path
                        if kt == 0:
                            nc.vector.tensor_copy(pacc[:], ptile[:])
                        else:
                            nc.vector.tensor_add(out=pacc[:, q0:512],
                                                 in0=pacc[:, q0:512],
                                                 in1=ptile[:, :free])
                        if vstep is not None and kt % 2 == 1:
                            vstep()
                    nc.scalar.copy(onorm[:, h, :], po[:])
                    pending_norm.append((pacc, onorm, h))
                    if len(pending_norm) > 1:
                        emit_norm(*pending_norm.pop(0))

                pending_y.append((onorm, b, qt))
                if len(pending_y) > 1:
                    emit_yproj(*pending_y.pop(0))

            # ---- schedule: attention unit (b,qt) = tile b*4+qt is emitted
            # mid-way through the NEXT tile's emission (see emit_tile) ----
            for tt in range(TT):
                if tt >= 1:
                    b, qt = divmod(tt - 1, QT)
                    emit_tile(tt, mid_cb=lambda vs, b=b, qt=qt: emit_attn(b, qt, vs))
                else:
                    emit_tile(tt)
                if tt == 1:
                    for h in range(HPC):
                        nc.scalar.dma_start(
                            wo_t[:, h, :],
                            woT.rearrange("(ko ki) n -> ki ko n", ki=P)[:, h, :])
            emit_attn(1, 3)
            for args in pending_norm:
                emit_norm(*args)
            for args in pending_y:
                emit_yproj(*args)

    nc.compile()
    return nc


def get_nc():
    if "nc" not in _CACHE:
        _CACHE["nc"] = _build_nc()
    return _CACHE["nc"]


def make_in_maps(x, cos, sin, wq, wk, wv, wo):
    bf = ml_dtypes.bfloat16
    xT = x.reshape(TOK, D).T  # [D, TOK]
    # [D, TOK] -> [TT, cpair, ci, j, 512]
    xTt = np.ascontiguousarray(
        xT.reshape(CPAIRS, 2, P, TT, 512).transpose(3, 0, 2, 1, 4)).astype(bf)
    cosT = np.ascontiguousarray(cos.reshape(TOK, HD).T).astype(bf)
    # rows 0:64 negated: rot_half contributes -x2*sin there (see kernel RoPE)
    sinT = np.ascontiguousarray(sin.reshape(TOK, HD).T).copy()
    sinT[0:64, :] *= -1.0
    sinT = sinT.astype(bf)
    in_maps = []
    for c in range(NCORES):
        dsl = slice(c * DC, (c + 1) * DC)
        in_maps.append({
            "xTt": xTt,
            "cosT": cosT,
            "sinT": sinT,
            "wqT": np.ascontiguousarray(wq[dsl, :].T).astype(bf),
            "wkT": np.ascontiguousarray(wk[dsl, :].T).astype(bf),
            "wvT": np.ascontiguousarray(wv[dsl, :].T).astype(bf),
            "woT": np.ascontiguousarray(wo[:, dsl].T).astype(bf),
        })
    return in_maps


def kernel(x, cos, sin, wq, wk, wv, wo):
    from concourse.bass_utils import run_bass_kernel_spmd

    nc = get_nc()
    in_maps = make_in_maps(
        np.asarray(x, dtype=np.float32), np.asarray(cos, dtype=np.float32),
        np.asarray(sin, dtype=np.float32), np.asarray(wq, dtype=np.float32),
        np.asarray(wk, dtype=np.float32), np.asarray(wv, dtype=np.float32),
        np.asarray(wo, dtype=np.float32))
    res = run_bass_kernel_spmd(nc, in_maps, list(range(NCORES)))
    out = np.zeros((TOK, D), dtype=np.float64)
    for m in res.results:
        out += m["y"].astype(np.float64)
    return out.astype(np.float32).reshape(B, T, D)
